# revision 21
# baseline (speedup 1.0000x reference)
"""Trainium2 Bass kernel for a dense transformer encoder layer.

Problem: B=4, S=2048, D=768, H=12 heads (DH=64), FFN 3072, fp32 I/O.

Sharding (no collectives): 8 cores = (batch b, sequence half) pairs.
Each core computes the full layer for its 1024 query rows; K/V projections
for the full 2048-row sequence of its batch are duplicated across the two
cores sharing a batch (cheaper than collectives here).

v2 changes vs v1:
- Softmax denominator folded into the ctx matmul: the V stationary carries a
  65th ones-column, so the PSUM ctx tile row 64 accumulates sum(exp) for
  free (eliminates all den matmuls).
- fp8 e4m3 DoubleRow matmuls (256-deep contraction per instruction) for the
  Q/K/V projections, attention ctx, and out-projection. Weights are scaled
  x16 on the host so fp8 stays in normal range; descale is folded into the
  existing scale slots. Attention output is ~0.7% of the residual stream,
  so fp8 errors there are diluted ~150x in the final output.
- Projection PSUM->SBUF writes moved from the Activation engine to DVE
  (tensor_scalar with scale+bias slots) so the scalar engine is dedicated
  to the 25M-element exp stream.
- ctx/ctxS kept per-head in 64 partitions; Wo rows are consumed in natural
  (pair, sub, 64) order so no host permutation is needed.

FFN stays float32r (fp8 would blow the error budget there).
"""
from contextlib import ExitStack

import numpy as np
import ml_dtypes

import concourse.bass as bass
import concourse.tile as tile
from concourse import bacc, mybir
from concourse.bass_utils import run_bass_kernel_spmd

FR = mybir.dt.float32r
F32 = mybir.dt.float32
BF = mybir.dt.bfloat16
F8 = mybir.dt.float8e4
AF = mybir.ActivationFunctionType
OP = mybir.AluOpType
DR = mybir.MatmulPerfMode.DoubleRow

B, S, D, H = 4, 2048, 768, 12
DH, DF = 64, 3072
SQ = 1024            # query rows per core
NK = D // 128        # 6 feature chunks
NP = D // 256        # 3 feature DoubleRow pairs
NF = DF // 128       # 24 ffn chunks
KC = S // 128        # 16 key chunks
KT = KC // 2         # 8 key DoubleRow pairs
NQ = SQ // 512       # 2 query column chunks
HP = H // 2          # 6 head pairs
NT = 8               # FFN weight slices
MF = 3               # dF 128-chunks per slice
DT = DF // NT        # 384 cols per W1 slice
N_CORES = 8
SCALE = 1.0 / 8.0    # 1/sqrt(DH)
EPS = 1e-5
Q16 = 1.0 / 16.0     # fp8 weight descale
QO = 1.0 / 1024.0    # out-proj descale (64 softmax-scale * 16 weight-scale)

GELU_FUNC = AF.Gelu     # test_sim swaps to Identity (CoreSim lacks Gelu)

# bias pack layout (columns in "sp" [128, 828])
_BQ, _BK, _BO, _B2, _LNG, _LNB, _B1, _BV = 0, 6, 12, 18, 24, 30, 36, 60


def _body(nc, tc, io):
    xq8_d, xqf_d, xk8_d, xv8_d = io["xq8"], io["xqf"], io["xk8"], io["xv8"]
    wq_d, wk_d, wv_d, wo_d = io["wq8"], io["wk8"], io["wv8"], io["wo8"]
    w1_d, w2_d, sp_d = io["w1"], io["w2"], io["sp"]
    ones_fr_d, out_d = io["ones_fr"], io["out"]

    r6 = lambda ap: ap.rearrange("(c p) s -> p c s", p=128)
    # fp8 inputs/weights: DRAM rows (pair t, sub i, partition p)
    r8x = lambda ap: ap.rearrange("(t i p) s -> p t i s", p=128, i=2)

    with ExitStack() as ctx:
        Po = lambda **kw: ctx.enter_context(tc.tile_pool(**kw))
        const = Po(name="const", bufs=1)
        sb = Po(name="sb", bufs=1)

        sp = const.tile([128, 828], F32)
        nc.sync.dma_start(out=sp[:], in_=sp_d)
        ones_fr = const.tile([128, 128], FR)
        nc.sync.dma_start(out=ones_fr[:], in_=ones_fr_d)
        sel65 = const.tile([128, 64], FR)
        nc.sync.dma_start(out=sel65[:], in_=io["sel65"])
        rr = const.tile([65, 1024], FR)
        nc.sync.dma_start(out=rr[:], in_=io["ones65"])
        bias = lambda idx, j: sp[:, idx + j : idx + j + 1]

        # shared weight slots: 4 x 9KB
        def wtile(name, shape, dt):
            return sb.tile(shape, dt, tag="w", bufs=4, name=name)

        wk8 = wtile("wk8", [128, NP * 2 * D], F8)
        nc.sync.dma_start(
            out=wk8[:].rearrange("p (t i m) -> p t i m", i=2, m=D), in_=r8x(wk_d))
        wq8 = wtile("wq8", [128, NP * 2 * D], F8)
        nc.sync.dma_start(
            out=wq8[:].rearrange("p (t i m) -> p t i m", i=2, m=D), in_=r8x(wq_d))
        wv8 = wtile("wv8", [128, NP * 2 * D], F8)
        nc.sync.dma_start(
            out=wv8[:].rearrange("p (t i m) -> p t i m", i=2, m=D), in_=r8x(wv_d))
        wk8r = wk8[:].rearrange("p (t i m) -> p t i m", i=2, m=D)
        wq8r = wq8[:].rearrange("p (t i m) -> p t i m", i=2, m=D)
        wv8r = wv8[:].rearrange("p (t i m) -> p t i m", i=2, m=D)

        # persistent activations (tag overlays: kpT->x_sb, vp8->hT)
        kpT = sb.tile([128, NK * S], BF, tag="kpx", name="kpT")
        qpT = sb.tile([128, NK * SQ], BF, tag="qpT", name="qpT")
        # V stationary blocks [128, 2, 128]: cols 0:64 = V features, col 64 =
        # ones (softmax denominator), cols 65:128 zero padding (ldweights
        # dual-fp8 requires 64/128-col stationaries; padding costs no cycles).
        vp8 = sb.tile([128, KT * H * 2 * 128], F8, tag="vph", name="vp8")
        vp8r = vp8[:].rearrange("p (t h i d) -> p t h i d", h=H, i=2, d=128)
        ctxS8 = sb.tile([64, H * SQ], F8, tag="ctxS", name="ctxS8")
        ctxS8r = ctxS8[:].rearrange("p (u i s) -> p u i s", i=2, s=SQ)

        # denominator ones column + zero padding of each vp8 block
        nc.gpsimd.memset(vp8r[:, :, :, :, 64:65], 1.0)
        nc.gpsimd.memset(vp8r[:, :, :, :, 65:128], 0.0)

        # ---------------- phase A: projections (fp8 DoubleRow) ----------------
        with tc.tile_pool(name="pa", bufs=4, space="PSUM") as pa:
            # kpT[mc, s] = sum_t Wk[t,:,mc].T @ xk[t, s]  (x16, descale+bk on DVE)
            for sc in range(S // 512):
                xk_t = sb.tile([128, NP * 2 * 512], F8, tag="xu", bufs=3, name="xk_t")
                xk_tr = xk_t[:].rearrange("p (t i s) -> p t i s", i=2, s=512)
                nc.sync.dma_start(
                    out=xk_tr, in_=r8x(xk8_d)[:, :, :, sc * 512 : (sc + 1) * 512])
                for mc in range(NK):
                    ps = pa.tile([128, 512], F32, tag="pa", name="psk")
                    for t in range(NP):
                        nc.tensor.matmul(
                            ps[:],
                            wk8r[:, t, :, mc * 128 : (mc + 1) * 128],
                            xk_tr[:, t],
                            start=(t == 0), stop=(t == NP - 1), perf_mode=DR)
                    with nc.allow_low_precision(reason="bf16 kpT"):
                        nc.vector.tensor_scalar(
                            kpT[:, mc * S + sc * 512 : mc * S + (sc + 1) * 512],
                            ps[:], Q16, bias(_BK, mc), OP.mult, OP.add)

            # qpT likewise (+bq)
            for sc in range(NQ):
                xq_t = sb.tile([128, NP * 2 * 512], F8, tag="xu", bufs=3, name="xq_t")
                xq_tr = xq_t[:].rearrange("p (t i s) -> p t i s", i=2, s=512)
                nc.sync.dma_start(
                    out=xq_tr, in_=r8x(xq8_d)[:, :, :, sc * 512 : (sc + 1) * 512])
                for mc in range(NK):
                    ps = pa.tile([128, 512], F32, tag="pa", name="psq")
                    for t in range(NP):
                        nc.tensor.matmul(
                            ps[:],
                            wq8r[:, t, :, mc * 128 : (mc + 1) * 128],
                            xq_tr[:, t],
                            start=(t == 0), stop=(t == NP - 1), perf_mode=DR)
                    with nc.allow_low_precision(reason="bf16 qpT"):
                        nc.vector.tensor_scalar(
                            qpT[:, mc * SQ + sc * 512 : mc * SQ + (sc + 1) * 512],
                            ps[:], Q16, bias(_BQ, mc), OP.mult, OP.add)

            # vp8[seq-chunk, (t,h,i,dh)] = xv[t, seq].T @ Wv[t, d]  (+bv)
            for sc in range(S // 512):
                xv_t = sb.tile([128, NP * 2 * 512], F8, tag="xu", bufs=3, name="xv_t")
                xv_tr = xv_t[:].rearrange("p (t i s) -> p t i s", i=2, s=512)
                nc.sync.dma_start(
                    out=xv_tr, in_=r8x(xv8_d)[:, :, :, sc * 512 : (sc + 1) * 512])
                for m in range(4):
                    srow = sc * 4 + m               # 128-row key chunk index
                    tp, si = divmod(srow, 2)        # DoubleRow pair, sub index
                    for n0, nsz in ((0, 512), (512, 256)):
                        ps = pa.tile([128, 512], F32, tag="pa", name="psv")
                        for t in range(NP):
                            nc.tensor.matmul(
                                ps[:, :nsz],
                                xv_tr[:, t, :, m * 128 : (m + 1) * 128],
                                wv8r[:, t, :, n0 : n0 + nsz],
                                start=(t == 0), stop=(t == NP - 1), perf_mode=DR)
                        nh = nsz // 64
                        h0 = n0 // 64
                        with nc.allow_low_precision(reason="fp8 vp"):
                            nc.vector.scalar_tensor_tensor(
                                vp8r[:, tp, h0 : h0 + nh, si, 0:64],
                                ps[:, :nsz].rearrange("p (h d) -> p h d", d=64),
                                Q16,
                                sp[:, _BV + n0 : _BV + n0 + nsz].rearrange(
                                    "p (h d) -> p h d", d=64),
                                OP.mult, OP.add)

        # ---------------- phase B: attention ----------------
        wo8 = wtile("wo8", [64, NK * 2 * D], F8)
        nc.sync.dma_start(
            out=wo8[:].rearrange("p (u i m) -> p u i m", i=2, m=D),
            in_=wo_d.rearrange("(u i p) m -> p u i m", p=64, i=2))
        wo8r = wo8[:].rearrange("p (u i m) -> p u i m", i=2, m=D)

        with (tc.tile_pool(name="sc_ps", bufs=2, space="PSUM") as scp,
              tc.tile_pool(name="cxa_ps", bufs=1, space="PSUM") as cxa,
              tc.tile_pool(name="cxb_ps", bufs=1, space="PSUM") as cxb,
              tc.tile_pool(name="rb_ps", bufs=2, space="PSUM") as rbp):
            for qc in range(NQ):
                for j in range(HP):
                    hA, hB = 2 * j, 2 * j + 1
                    ctxA = cxa.tile([128, 512], F32, tag="cxa", name="ctxA")
                    ctxB = cxb.tile([128, 512], F32, tag="cxb", name="ctxB")
                    for t in range(KT):
                        eAB8 = sb.tile([128, 2048], F8, tag="ex", bufs=4,
                                       name="eAB8")
                        e8r = eAB8[:].rearrange("p (i x) -> p i x", i=2)
                        for i in range(2):
                            kc = 2 * t + i
                            psAB = scp.tile([128, 1024], F32, tag="sc",
                                            name="psAB")
                            nc.tensor.matmul(
                                psAB[:, 0:512],
                                kpT[0:64, j * S + kc * 128 : j * S + (kc + 1) * 128],
                                qpT[0:64, j * SQ + qc * 512 : j * SQ + (qc + 1) * 512],
                                start=True, stop=True)
                            nc.tensor.matmul(
                                psAB[:, 512:1024],
                                kpT[64:128, j * S + kc * 128 : j * S + (kc + 1) * 128],
                                qpT[64:128, j * SQ + qc * 512 : j * SQ + (qc + 1) * 512],
                                start=True, stop=True, skip_group_check=True)
                            with nc.allow_low_precision(reason="fp8 exp"):
                                nc.scalar.activation(
                                    eAB8[:, i * 1024 : (i + 1) * 1024],
                                    psAB[:], AF.Exp, scale=SCALE)
                        nc.tensor.matmul(
                            ctxA[:],
                            vp8r[:, t, hA],
                            e8r[:, :, 0:512],
                            start=(t == 0), stop=(t == KT - 1),
                            perf_mode=DR, skip_group_check=True)
                        nc.tensor.matmul(
                            ctxB[:],
                            vp8r[:, t, hB],
                            e8r[:, :, 512:1024],
                            start=(t == 0), stop=(t == KT - 1),
                            perf_mode=DR, skip_group_check=True)

                    # normalize: row 64 of ctxA/ctxB is sum(exp); rb = 64/den
                    with nc.allow_low_precision(reason="softmax recip"):
                        nc.vector.reciprocal(rr[64:65, 0:512], ctxA[64:65, :])
                        nc.vector.reciprocal(rr[64:65, 512:1024], ctxB[64:65, :])
                    rbA = rbp.tile([64, 512], F32, tag="rb", name="rbA")
                    rbB = rbp.tile([64, 512], F32, tag="rb", name="rbB")
                    nc.tensor.matmul(rbA[:], sel65[0:65, :], rr[0:65, 0:512],
                                     start=True, stop=True)
                    nc.tensor.matmul(rbB[:], sel65[0:65, :], rr[0:65, 512:1024],
                                     start=True, stop=True, skip_group_check=True)
                    # walrus rejects 2-PSUM-input TensorTensor: stage raw ctx
                    # in SBUF fp8, then multiply by the PSUM recip broadcast.
                    ccA = sb.tile([64, 512], F8, tag="cc", bufs=2, name="ccA")
                    ccB = sb.tile([64, 512], F8, tag="cc", bufs=2, name="ccB")
                    with nc.allow_low_precision(reason="fp8 ctxS"):
                        nc.vector.tensor_copy(ccA[:], ctxA[0:64, :])
                        nc.vector.tensor_copy(ccB[:], ctxB[0:64, :])
                        nc.vector.tensor_mul(
                            ctxS8[0:64, hA * SQ + qc * 512 : hA * SQ + (qc + 1) * 512],
                            ccA[:], rbA[:])
                        nc.vector.tensor_mul(
                            ctxS8[0:64, hB * SQ + qc * 512 : hB * SQ + (qc + 1) * 512],
                            ccB[:], rbB[:])

        # ------ phase C: out-proj + residual + LayerNorm (interleaved) ------
        # The LN stats matmuls (mean/var) consume each x_sb chunk right after
        # the out-projection writes it, instead of a separate re-scan phase:
        # collapses the C1->C2 serial chain. PSUM: pc 2 + st 4 + ab 2 = 8.
        x_sb = sb.tile([128, NK * SQ], FR, tag="kpx", name="x_sb")
        hT = sb.tile([128, NK * SQ], FR, tag="vph", name="hT")
        with (tc.tile_pool(name="pc", bufs=2, space="PSUM") as pc,
              tc.tile_pool(name="st_ps", bufs=4, space="PSUM") as st_ps,
              tc.tile_pool(name="ab_ps", bufs=2, space="PSUM") as ab_ps):
            mean_p, var_p = {}, {}
            for qc in range(NQ):
                mean_p[qc] = st_ps.tile([1, 512], F32, tag="st", name=f"mean{qc}")
                var_p[qc] = st_ps.tile([1, 512], F32, tag="st", name=f"var{qc}")
            for qc in range(NQ):
                for mc in range(NK):
                    ps = pc.tile([128, 512], F32, tag="pc", name="pso")
                    for u in range(NK):
                        nc.tensor.matmul(
                            ps[:],
                            wo8r[:, u, :, mc * 128 : (mc + 1) * 128],
                            ctxS8r[:, u, :, qc * 512 : (qc + 1) * 512],
                            start=(u == 0), stop=(u == NK - 1), perf_mode=DR)
                    xqc = sb.tile([128, 512], F32, tag="sm", bufs=2, name="xqc")
                    nc.sync.dma_start(
                        out=xqc[:],
                        in_=r6(xqf_d)[:, mc, qc * 512 : (qc + 1) * 512])
                    x_sl = x_sb[:, mc * SQ + qc * 512 : mc * SQ + (qc + 1) * 512]
                    with nc.allow_low_precision(reason="f32r residual"):
                        nc.vector.scalar_tensor_tensor(
                            x_sl, ps[:], QO, xqc[:], OP.mult, OP.add)
                    xsq = sb.tile([128, 512], FR, tag="sq", bufs=2, name="xsq")
                    with nc.allow_low_precision(reason="f32r x^2 for LN var"):
                        nc.vector.tensor_mul(xsq[:], x_sl, x_sl)
                    nc.tensor.matmul(
                        mean_p[qc][:], ones_fr[:, 0:1], x_sl,
                        start=(mc == 0), stop=(mc == NK - 1),
                        skip_group_check=True)
                    nc.tensor.matmul(
                        var_p[qc][:], ones_fr[:, 0:1], xsq[:],
                        start=(mc == 0), stop=(mc == NK - 1),
                        skip_group_check=True)
                mu = sb.tile([1, 512], F32, tag="r1", bufs=2, name="mu")
                e2 = sb.tile([1, 512], F32, tag="r2", bufs=2, name="e2")
                sd = sb.tile([1, 512], F32, tag="r3", bufs=2, name="sd")
                rs = sb.tile([1, 512], FR, tag="r4", bufs=2, name="rs")
                mrs = sb.tile([1, 512], FR, tag="r5", bufs=2, name="mrs")
                nc.vector.tensor_scalar_mul(mu[:], mean_p[qc][:], 1.0 / D)
                nc.vector.tensor_scalar_mul(e2[:], var_p[qc][:], 1.0 / D)
                nc.vector.tensor_mul(sd[:], mu[:], mu[:])
                nc.vector.tensor_sub(e2[:], e2[:], sd[:])        # variance
                nc.vector.tensor_scalar_add(e2[:], e2[:], EPS)
                nc.scalar.activation(sd[:], e2[:], AF.Sqrt)
                with nc.allow_low_precision(reason="f32r LN rows"):
                    nc.vector.reciprocal(rs[:], sd[:])
                    nc.vector.tensor_mul(mrs[:], mu[:].bitcast(FR), rs[:])
                A_p = ab_ps.tile([128, 512], F32, tag="ab", name="A_p")
                B_p = ab_ps.tile([128, 512], F32, tag="ab", name="B_p")
                nc.tensor.matmul(A_p[:], ones_fr[0:1, :], rs[:],
                                 start=True, stop=True)
                nc.tensor.matmul(B_p[:], ones_fr[0:1, :], mrs[:],
                                 start=True, stop=True)
                for kc in range(NK):
                    t1 = sb.tile([128, 512], F32, tag="sm", bufs=2, name="t1")
                    nc.vector.tensor_mul(
                        t1[:],
                        x_sb[:, kc * SQ + qc * 512 : kc * SQ + (qc + 1) * 512],
                        A_p[:])
                    t2 = sb.tile([128, 512], F32, tag="sm", bufs=2, name="t2")
                    nc.vector.tensor_sub(t2[:], t1[:], B_p[:])
                    with nc.allow_low_precision(reason="f32r hT"):
                        nc.scalar.activation(
                            hT[:, kc * SQ + qc * 512 : kc * SQ + (qc + 1) * 512],
                            t2[:], AF.Identity,
                            bias=bias(_LNB, kc), scale=bias(_LNG, kc))

        # ---------------- phase D: FFN ----------------
        # W1/W2 slices loaded ONCE per rep (t-outer); per-t partial outputs
        # accumulate into SBUF via DVE so PSUM stays at 4 banks and weight
        # DMA halves vs the sc-outer variant.
        acc = sb.tile([128, NK * SQ], F32, tag="acc", name="acc")
        with (tc.tile_pool(name="ff_ps", bufs=2, space="PSUM") as ff_ps,
              tc.tile_pool(name="u_ps", bufs=2, space="PSUM") as u_ps):
            for t in range(NT):
                w1t = wtile(f"w1_{t}", [128, NK * DT], FR)
                nc.sync.dma_start(
                    out=w1t[:].rearrange("p (c m) -> p c m", m=DT),
                    in_=w1_d.rearrange("(c p) (t m) -> p c t m",
                                       p=128, t=NT)[:, :, t, :])
                w2t = wtile(f"w2_{t}", [128, MF * D], FR)
                nc.sync.dma_start(
                    out=w2t[:].rearrange("p (c m) -> p c m", m=D),
                    in_=w2_d.rearrange("(t c p) m -> p t c m",
                                       p=128, c=MF)[:, t])
                for sc in range(NQ):
                    ut = sb.tile([128, MF * 512], FR, tag="xu2", bufs=2, name="ut")
                    for mf in range(MF):
                        up = u_ps.tile([128, 512], F32, tag="up", name="up")
                        for kc in range(NK):
                            nc.tensor.matmul(
                                up[:],
                                w1t[:, kc * DT + mf * 128 : kc * DT + (mf + 1) * 128],
                                hT[:, kc * SQ + sc * 512 : kc * SQ + (sc + 1) * 512],
                                start=(kc == 0), stop=(kc == NK - 1))
                        with nc.allow_low_precision(reason="f32r u"):
                            nc.scalar.activation(
                                ut[:, mf * 512 : (mf + 1) * 512], up[:],
                                GELU_FUNC, bias=bias(_B1, t * MF + mf),
                                scale=1.0)
                    for md in range(NK):
                        ffp = ff_ps.tile([128, 512], F32, tag="ff", name="ffp")
                        for c in range(MF):
                            nc.tensor.matmul(
                                ffp[:],
                                w2t[:, c * D + md * 128 : c * D + (md + 1) * 128],
                                ut[:, c * 512 : (c + 1) * 512],
                                start=(c == 0), stop=(c == MF - 1))
                        a_sl = acc[:, md * SQ + sc * 512 : md * SQ + (sc + 1) * 512]
                        if t == 0:
                            nc.vector.tensor_copy(a_sl, ffp[:])
                        else:
                            nc.vector.tensor_add(a_sl, a_sl, ffp[:])
            for sc in range(NQ):
                for md in range(NK):
                    ot = sb.tile([128, 512], F32, tag="sm", bufs=2, name="ot")
                    nc.scalar.activation(
                        ot[:],
                        acc[:, md * SQ + sc * 512 : md * SQ + (sc + 1) * 512],
                        AF.Identity, bias=bias(_B2, md), scale=1.0)
                    nc.sync.dma_start(
                        out=r6(out_d)[:, md, sc * 512 : (sc + 1) * 512],
                        in_=ot[:])


def _build(reps=1):
    nc = bacc.Bacc("TRN2", target_bir_lowering=False, debug=False,
                   num_devices=N_CORES)
    io = {
        "xq8": nc.dram_tensor("xq8", [D, SQ], F8, kind="ExternalInput").ap(),
        "xqf": nc.dram_tensor("xqf", [D, SQ], F32, kind="ExternalInput").ap(),
        "xk8": nc.dram_tensor("xk8", [D, S], F8, kind="ExternalInput").ap(),
        "xv8": nc.dram_tensor("xv8", [D, S], F8, kind="ExternalInput").ap(),
        "wq8": nc.dram_tensor("wq8", [D, D], F8, kind="ExternalInput").ap(),
        "wk8": nc.dram_tensor("wk8", [D, D], F8, kind="ExternalInput").ap(),
        "wv8": nc.dram_tensor("wv8", [D, D], F8, kind="ExternalInput").ap(),
        "wo8": nc.dram_tensor("wo8", [D, D], F8, kind="ExternalInput").ap(),
        "sel65": nc.dram_tensor("sel65", [128, 64], FR,
                                kind="ExternalInput").ap(),
        "ones65": nc.dram_tensor("ones65", [65, 1024], FR,
                                 kind="ExternalInput").ap(),
        "w1": nc.dram_tensor("w1", [D, DF], FR, kind="ExternalInput").ap(),
        "w2": nc.dram_tensor("w2", [DF, D], FR, kind="ExternalInput").ap(),
        "sp": nc.dram_tensor("sp", [128, 828], F32, kind="ExternalInput").ap(),
        "ones_fr": nc.dram_tensor("ones_fr", [128, 128], FR,
                                  kind="ExternalInput").ap(),
        "out": nc.dram_tensor("out", [D, SQ], F32, kind="ExternalOutput").ap(),
    }
    with tile.TileContext(nc) as tc:
        if reps == 1:
            _body(nc, tc, io)
        else:
            with tc.For_i(0, reps, 1):
                _body(nc, tc, io)
    nc.compile()
    return nc


_NC = None


def _get_nc():
    global _NC
    if _NC is None:
        _NC = _build()
    return _NC


def make_in_maps(inputs):
    """Shard + lay out the full inputs for the 8 cores (numpy only)."""
    f = lambda k: np.asarray(inputs[k], np.float32)
    f8 = lambda a: np.ascontiguousarray(a).astype(ml_dtypes.float8_e4m3)
    Q, K, V = f("Q"), f("K"), f("V")
    sp = np.zeros((128, 828), np.float32)
    for idx, key in ((_BQ, "bq"), (_BK, "bk"), (_B2, "b2"),
                     (_LNG, "ln_g"), (_LNB, "ln_b")):
        sp[:, idx : idx + NK] = f(key).reshape(NK, 128).T
    sp[:, _B1 : _B1 + NF] = f("b1").reshape(NF, 128).T
    sp[:, _BV : _BV + D] = np.broadcast_to(f("bv"), (128, D))
    sel65 = np.zeros((128, 64), np.float32)
    sel65[64, :] = 64.0
    shared = {
        "wq8": f8(16.0 * f("Wq")), "wk8": f8(16.0 * f("Wk")),
        "wv8": f8(16.0 * f("Wv")), "wo8": f8(16.0 * f("Wo")),
        "w1": f("W1"), "w2": f("W2"), "sp": sp,
        "ones_fr": np.ones((128, 128), np.float32),
        "sel65": sel65,
        "ones65": np.ones((65, 1024), np.float32),
    }
    bo = f("bo")
    in_maps = []
    for c in range(N_CORES):
        b, half = divmod(c, 2)
        r0 = half * SQ
        xq = np.ascontiguousarray(Q[b, r0 : r0 + SQ, :].T)
        in_maps.append(dict(
            shared,
            xq8=f8(xq),
            xqf=xq + bo[:, None],
            xk8=f8(K[b].T),
            xv8=f8(V[b].T),
        ))
    return in_maps


def assemble(results):
    out = np.empty((B, S, D), np.float32)
    for c in range(N_CORES):
        b, half = divmod(c, 2)
        r0 = half * SQ
        out[b, r0 : r0 + SQ, :] = results[c]["out"].T
    return out


def kernel(**inputs):
    nc = _get_nc()
    res = run_bass_kernel_spmd(nc, make_in_maps(inputs), list(range(N_CORES)))
    return assemble(res.results)


# revision 23
# speedup vs baseline: 1.1815x; 1.1815x over previous
"""Trainium2 Bass kernel for a dense transformer encoder layer.

Problem: B=4, S=2048, D=768, H=12 heads (DH=64), FFN 3072, fp32 I/O.

Sharding (no collectives): 8 cores = (batch b, sequence half) pairs.
Each core computes the full layer for its 1024 query rows; K/V projections
for the full 2048-row sequence of its batch are duplicated across the two
cores sharing a batch (cheaper than collectives here).

v2 changes vs v1:
- Softmax denominator folded into the ctx matmul: the V stationary carries a
  65th ones-column, so the PSUM ctx tile row 64 accumulates sum(exp) for
  free (eliminates all den matmuls).
- fp8 e4m3 DoubleRow matmuls (256-deep contraction per instruction) for the
  Q/K/V projections, attention ctx, and out-projection. Weights are scaled
  x16 on the host so fp8 stays in normal range; descale is folded into the
  existing scale slots. Attention output is ~0.7% of the residual stream,
  so fp8 errors there are diluted ~150x in the final output.
- Projection PSUM->SBUF writes moved from the Activation engine to DVE
  (tensor_scalar with scale+bias slots) so the scalar engine is dedicated
  to the 25M-element exp stream.
- ctx/ctxS kept per-head in 64 partitions; Wo rows are consumed in natural
  (pair, sub, 64) order so no host permutation is needed.

FFN stays float32r (fp8 would blow the error budget there).
"""
from contextlib import ExitStack

import numpy as np
import ml_dtypes

import concourse.bass as bass
import concourse.tile as tile
from concourse import bacc, mybir
from concourse.bass_utils import run_bass_kernel_spmd

FR = mybir.dt.float32r
F32 = mybir.dt.float32
BF = mybir.dt.bfloat16
F8 = mybir.dt.float8e4
AF = mybir.ActivationFunctionType
OP = mybir.AluOpType
DR = mybir.MatmulPerfMode.DoubleRow

B, S, D, H = 4, 2048, 768, 12
DH, DF = 64, 3072
SQ = 1024            # query rows per core
NK = D // 128        # 6 feature chunks
NP = D // 256        # 3 feature DoubleRow pairs
NF = DF // 128       # 24 ffn chunks
KC = S // 128        # 16 key chunks
KT = KC // 2         # 8 key DoubleRow pairs
NQ = SQ // 512       # 2 query column chunks
HP = H // 2          # 6 head pairs
NT = 8               # FFN weight slices
MF = 3               # dF 128-chunks per slice
DT = DF // NT        # 384 cols per W1 slice
N_CORES = 8
SCALE = 1.0 / 8.0    # 1/sqrt(DH)
EPS = 1e-5
Q16 = 1.0 / 16.0     # fp8 weight descale
QO = 1.0 / 1024.0    # out-proj descale (64 softmax-scale * 16 weight-scale)

GELU_FUNC = AF.Gelu     # test_sim swaps to Identity (CoreSim lacks Gelu)

# bias pack layout (columns in "sp" [128, 828])
_BQ, _BK, _BO, _B2, _LNG, _LNB, _B1, _BV = 0, 6, 12, 18, 24, 30, 36, 60


def _body(nc, tc, io):
    xq8_d, xqf_d, xk8_d, xv8_d = io["xq8"], io["xqf"], io["xk8"], io["xv8"]
    wq_d, wk_d, wv_d, wo_d = io["wq8"], io["wk8"], io["wv8"], io["wo8"]
    w1_d, w2_d, sp_d = io["w1"], io["w2"], io["sp"]
    ones_fr_d, out_d = io["ones_fr"], io["out"]

    r6 = lambda ap: ap.rearrange("(c p) s -> p c s", p=128)
    # fp8 inputs/weights: DRAM rows (pair t, sub i, partition p)
    r8x = lambda ap: ap.rearrange("(t i p) s -> p t i s", p=128, i=2)

    with ExitStack() as ctx:
        Po = lambda **kw: ctx.enter_context(tc.tile_pool(**kw))
        const = Po(name="const", bufs=1)
        sb = Po(name="sb", bufs=1)

        sp = const.tile([128, 828], F32)
        nc.sync.dma_start(out=sp[:], in_=sp_d)
        ones_fr = const.tile([128, 128], FR)
        nc.sync.dma_start(out=ones_fr[:], in_=ones_fr_d)
        sel65 = const.tile([128, 64], FR)
        nc.sync.dma_start(out=sel65[:], in_=io["sel65"])
        rr = const.tile([65, 1024], FR)
        nc.sync.dma_start(out=rr[:], in_=io["ones65"])
        bias = lambda idx, j: sp[:, idx + j : idx + j + 1]

        # shared weight slots: 4 x 9KB
        def wtile(name, shape, dt):
            return sb.tile(shape, dt, tag="w", bufs=4, name=name)

        wk8 = wtile("wk8", [128, NP * 2 * D], F8)
        nc.sync.dma_start(
            out=wk8[:].rearrange("p (t i m) -> p t i m", i=2, m=D), in_=r8x(wk_d))
        wq8 = wtile("wq8", [128, NP * 2 * D], F8)
        nc.sync.dma_start(
            out=wq8[:].rearrange("p (t i m) -> p t i m", i=2, m=D), in_=r8x(wq_d))
        wv8 = wtile("wv8", [128, NP * 2 * D], F8)
        nc.sync.dma_start(
            out=wv8[:].rearrange("p (t i m) -> p t i m", i=2, m=D), in_=r8x(wv_d))
        wk8r = wk8[:].rearrange("p (t i m) -> p t i m", i=2, m=D)
        wq8r = wq8[:].rearrange("p (t i m) -> p t i m", i=2, m=D)
        wv8r = wv8[:].rearrange("p (t i m) -> p t i m", i=2, m=D)

        # persistent activations (tag overlays: kpT->x_sb, vp8->hT)
        kpT = sb.tile([128, NK * S], BF, tag="kpx", name="kpT")
        qpT = sb.tile([128, NK * SQ], BF, tag="qpT", name="qpT")
        # V stationary blocks [128, 2, 128]: cols 0:64 = V features, col 64 =
        # ones (softmax denominator), cols 65:128 zero padding (ldweights
        # dual-fp8 requires 64/128-col stationaries; padding costs no cycles).
        vp8 = sb.tile([128, KT * H * 2 * 128], F8, tag="vph", name="vp8")
        vp8r = vp8[:].rearrange("p (t h i d) -> p t h i d", h=H, i=2, d=128)
        ctxS8 = sb.tile([64, H * SQ], F8, tag="ctxS", name="ctxS8")
        ctxS8r = ctxS8[:].rearrange("p (u i s) -> p u i s", i=2, s=SQ)

        # denominator ones column + zero padding of each vp8 block
        nc.gpsimd.memset(vp8r[:, :, :, :, 64:65], 1.0)
        nc.gpsimd.memset(vp8r[:, :, :, :, 65:128], 0.0)

        # ---------------- phase A: projections (fp8 DoubleRow) ----------------
        with tc.tile_pool(name="pa", bufs=4, space="PSUM") as pa:
            # kpT[mc, s] = sum_t Wk[t,:,mc].T @ xk[t, s]  (x16, descale+bk on DVE)
            for sc in range(S // 512):
                xk_t = sb.tile([128, NP * 2 * 512], F8, tag="xu", bufs=3, name="xk_t")
                xk_tr = xk_t[:].rearrange("p (t i s) -> p t i s", i=2, s=512)
                nc.sync.dma_start(
                    out=xk_tr, in_=r8x(xk8_d)[:, :, :, sc * 512 : (sc + 1) * 512])
                for mc in range(NK):
                    ps = pa.tile([128, 512], F32, tag="pa", name="psk")
                    for t in range(NP):
                        nc.tensor.matmul(
                            ps[:],
                            wk8r[:, t, :, mc * 128 : (mc + 1) * 128],
                            xk_tr[:, t],
                            start=(t == 0), stop=(t == NP - 1), perf_mode=DR)
                    with nc.allow_low_precision(reason="bf16 kpT"):
                        nc.vector.tensor_scalar(
                            kpT[:, mc * S + sc * 512 : mc * S + (sc + 1) * 512],
                            ps[:], Q16, bias(_BK, mc), OP.mult, OP.add)

            # qpT likewise (+bq)
            for sc in range(NQ):
                xq_t = sb.tile([128, NP * 2 * 512], F8, tag="xu", bufs=3, name="xq_t")
                xq_tr = xq_t[:].rearrange("p (t i s) -> p t i s", i=2, s=512)
                nc.sync.dma_start(
                    out=xq_tr, in_=r8x(xq8_d)[:, :, :, sc * 512 : (sc + 1) * 512])
                for mc in range(NK):
                    ps = pa.tile([128, 512], F32, tag="pa", name="psq")
                    for t in range(NP):
                        nc.tensor.matmul(
                            ps[:],
                            wq8r[:, t, :, mc * 128 : (mc + 1) * 128],
                            xq_tr[:, t],
                            start=(t == 0), stop=(t == NP - 1), perf_mode=DR)
                    with nc.allow_low_precision(reason="bf16 qpT"):
                        nc.vector.tensor_scalar(
                            qpT[:, mc * SQ + sc * 512 : mc * SQ + (sc + 1) * 512],
                            ps[:], Q16, bias(_BQ, mc), OP.mult, OP.add)

            # vp8[seq-chunk, (t,h,i,dh)] = xv[t, seq].T @ Wv[t, d]  (+bv)
            for sc in range(S // 512):
                xv_t = sb.tile([128, NP * 2 * 512], F8, tag="xu", bufs=3, name="xv_t")
                xv_tr = xv_t[:].rearrange("p (t i s) -> p t i s", i=2, s=512)
                nc.sync.dma_start(
                    out=xv_tr, in_=r8x(xv8_d)[:, :, :, sc * 512 : (sc + 1) * 512])
                for m in range(4):
                    srow = sc * 4 + m               # 128-row key chunk index
                    tp, si = divmod(srow, 2)        # DoubleRow pair, sub index
                    for n0, nsz in ((0, 512), (512, 256)):
                        ps = pa.tile([128, 512], F32, tag="pa", name="psv")
                        for t in range(NP):
                            nc.tensor.matmul(
                                ps[:, :nsz],
                                xv_tr[:, t, :, m * 128 : (m + 1) * 128],
                                wv8r[:, t, :, n0 : n0 + nsz],
                                start=(t == 0), stop=(t == NP - 1), perf_mode=DR)
                        nh = nsz // 64
                        h0 = n0 // 64
                        with nc.allow_low_precision(reason="fp8 vp"):
                            nc.vector.scalar_tensor_tensor(
                                vp8r[:, tp, h0 : h0 + nh, si, 0:64],
                                ps[:, :nsz].rearrange("p (h d) -> p h d", d=64),
                                Q16,
                                sp[:, _BV + n0 : _BV + n0 + nsz].rearrange(
                                    "p (h d) -> p h d", d=64),
                                OP.mult, OP.add)

        # ---------------- phase B: attention ----------------
        wo8 = wtile("wo8", [64, NK * 2 * D], F8)
        nc.sync.dma_start(
            out=wo8[:].rearrange("p (u i m) -> p u i m", i=2, m=D),
            in_=wo_d.rearrange("(u i p) m -> p u i m", p=64, i=2))
        wo8r = wo8[:].rearrange("p (u i m) -> p u i m", i=2, m=D)

        with (tc.tile_pool(name="sc_ps", bufs=2, space="PSUM") as scp,
              tc.tile_pool(name="cxa_ps", bufs=1, space="PSUM") as cxa,
              tc.tile_pool(name="cxb_ps", bufs=1, space="PSUM") as cxb,
              tc.tile_pool(name="rb_ps", bufs=2, space="PSUM") as rbp):
            for qc in range(NQ):
                for j in range(HP):
                    hA, hB = 2 * j, 2 * j + 1
                    ctxA = cxa.tile([128, 512], F32, tag="cxa", name="ctxA")
                    ctxB = cxb.tile([128, 512], F32, tag="cxb", name="ctxB")
                    for t in range(KT):
                        eAB8 = sb.tile([128, 2048], F8, tag="ex", bufs=4,
                                       name="eAB8")
                        e8r = eAB8[:].rearrange("p (i x) -> p i x", i=2)
                        for i in range(2):
                            kc = 2 * t + i
                            psAB = scp.tile([128, 1024], F32, tag="sc",
                                            name="psAB")
                            nc.tensor.matmul(
                                psAB[:, 0:512],
                                kpT[0:64, j * S + kc * 128 : j * S + (kc + 1) * 128],
                                qpT[0:64, j * SQ + qc * 512 : j * SQ + (qc + 1) * 512],
                                start=True, stop=True)
                            nc.tensor.matmul(
                                psAB[:, 512:1024],
                                kpT[64:128, j * S + kc * 128 : j * S + (kc + 1) * 128],
                                qpT[64:128, j * SQ + qc * 512 : j * SQ + (qc + 1) * 512],
                                start=True, stop=True, skip_group_check=True)
                            with nc.allow_low_precision(reason="fp8 exp"):
                                nc.scalar.activation(
                                    eAB8[:, i * 1024 : (i + 1) * 1024],
                                    psAB[:], AF.Exp, scale=SCALE)
                        nc.tensor.matmul(
                            ctxA[:],
                            vp8r[:, t, hA],
                            e8r[:, :, 0:512],
                            start=(t == 0), stop=(t == KT - 1),
                            perf_mode=DR, skip_group_check=True)
                        nc.tensor.matmul(
                            ctxB[:],
                            vp8r[:, t, hB],
                            e8r[:, :, 512:1024],
                            start=(t == 0), stop=(t == KT - 1),
                            perf_mode=DR, skip_group_check=True)

                    # normalize: row 64 of ctxA/ctxB is sum(exp); rb = 64/den
                    with nc.allow_low_precision(reason="softmax recip"):
                        nc.vector.reciprocal(rr[64:65, 0:512], ctxA[64:65, :])
                        nc.vector.reciprocal(rr[64:65, 512:1024], ctxB[64:65, :])
                    rbA = rbp.tile([64, 512], F32, tag="rb", name="rbA")
                    rbB = rbp.tile([64, 512], F32, tag="rb", name="rbB")
                    nc.tensor.matmul(rbA[:], sel65[0:65, :], rr[0:65, 0:512],
                                     start=True, stop=True)
                    nc.tensor.matmul(rbB[:], sel65[0:65, :], rr[0:65, 512:1024],
                                     start=True, stop=True, skip_group_check=True)
                    # walrus rejects 2-PSUM-input TensorTensor: stage raw ctx
                    # in SBUF fp8, then multiply by the PSUM recip broadcast.
                    ccA = sb.tile([64, 512], F8, tag="cc", bufs=2, name="ccA")
                    ccB = sb.tile([64, 512], F8, tag="cc", bufs=2, name="ccB")
                    with nc.allow_low_precision(reason="fp8 ctxS"):
                        nc.vector.tensor_copy(ccA[:], ctxA[0:64, :])
                        nc.vector.tensor_copy(ccB[:], ctxB[0:64, :])
                        nc.vector.tensor_mul(
                            ctxS8[0:64, hA * SQ + qc * 512 : hA * SQ + (qc + 1) * 512],
                            ccA[:], rbA[:])
                        nc.vector.tensor_mul(
                            ctxS8[0:64, hB * SQ + qc * 512 : hB * SQ + (qc + 1) * 512],
                            ccB[:], rbB[:])

        # ------------ phase C1: out-proj + residual (fp8 DoubleRow) ------------
        x_sb = sb.tile([128, NK * SQ], FR, tag="kpx", name="x_sb")
        with tc.tile_pool(name="pc", bufs=2, space="PSUM") as pc:
            for qc in range(NQ):
                for mc in range(NK):
                    ps = pc.tile([128, 512], F32, tag="pc", name="pso")
                    for u in range(NK):
                        nc.tensor.matmul(
                            ps[:],
                            wo8r[:, u, :, mc * 128 : (mc + 1) * 128],
                            ctxS8r[:, u, :, qc * 512 : (qc + 1) * 512],
                            start=(u == 0), stop=(u == NK - 1), perf_mode=DR)
                    xqc = sb.tile([128, 512], F32, tag="sm", bufs=2, name="xqc")
                    nc.sync.dma_start(
                        out=xqc[:],
                        in_=r6(xqf_d)[:, mc, qc * 512 : (qc + 1) * 512])
                    with nc.allow_low_precision(reason="f32r residual"):
                        nc.vector.scalar_tensor_tensor(
                            x_sb[:, mc * SQ + qc * 512 : mc * SQ + (qc + 1) * 512],
                            ps[:], QO, xqc[:], OP.mult, OP.add)

        # ---------------- phase C2: LayerNorm ----------------
        hT = sb.tile([128, NK * SQ], FR, tag="vph", name="hT")
        with (tc.tile_pool(name="st_ps", bufs=4, space="PSUM") as st_ps,
              tc.tile_pool(name="ab_ps", bufs=2, space="PSUM") as ab_ps):
            mean_p, var_p = {}, {}
            for qc in range(NQ):
                mean_p[qc] = st_ps.tile([1, 512], F32, tag="st", name=f"mean{qc}")
                var_p[qc] = st_ps.tile([1, 512], F32, tag="st", name=f"var{qc}")
            for kc in range(NK):
                xsq = sb.tile([128, SQ], FR, tag="sq", bufs=1, name="xsq")
                with nc.allow_low_precision(reason="f32r x^2 for LN var"):
                    nc.vector.tensor_mul(
                        xsq[:], x_sb[:, kc * SQ : (kc + 1) * SQ],
                        x_sb[:, kc * SQ : (kc + 1) * SQ])
                for qc in range(NQ):
                    nc.tensor.matmul(
                        mean_p[qc][:], ones_fr[:, 0:1],
                        x_sb[:, kc * SQ + qc * 512 : kc * SQ + (qc + 1) * 512],
                        start=(kc == 0), stop=(kc == NK - 1),
                        skip_group_check=True)
                    nc.tensor.matmul(
                        var_p[qc][:], ones_fr[:, 0:1],
                        xsq[:, qc * 512 : (qc + 1) * 512],
                        start=(kc == 0), stop=(kc == NK - 1),
                        skip_group_check=True)

            for qc in range(NQ):
                mu = sb.tile([1, 512], F32, tag="r1", bufs=2, name="mu")
                e2 = sb.tile([1, 512], F32, tag="r2", bufs=2, name="e2")
                sd = sb.tile([1, 512], F32, tag="r3", bufs=2, name="sd")
                rs = sb.tile([1, 512], FR, tag="r4", bufs=2, name="rs")
                mrs = sb.tile([1, 512], FR, tag="r5", bufs=2, name="mrs")
                nc.vector.tensor_scalar_mul(mu[:], mean_p[qc][:], 1.0 / D)
                nc.vector.tensor_scalar_mul(e2[:], var_p[qc][:], 1.0 / D)
                nc.vector.tensor_mul(sd[:], mu[:], mu[:])
                nc.vector.tensor_sub(e2[:], e2[:], sd[:])        # variance
                nc.vector.tensor_scalar_add(e2[:], e2[:], EPS)
                nc.scalar.activation(sd[:], e2[:], AF.Sqrt)
                with nc.allow_low_precision(reason="f32r LN rows"):
                    nc.vector.reciprocal(rs[:], sd[:])
                    nc.vector.tensor_mul(mrs[:], mu[:].bitcast(FR), rs[:])
                A_p = ab_ps.tile([128, 512], F32, tag="ab", name="A_p")
                B_p = ab_ps.tile([128, 512], F32, tag="ab", name="B_p")
                nc.tensor.matmul(A_p[:], ones_fr[0:1, :], rs[:],
                                 start=True, stop=True)
                nc.tensor.matmul(B_p[:], ones_fr[0:1, :], mrs[:],
                                 start=True, stop=True)
                for kc in range(NK):
                    t1 = sb.tile([128, 512], F32, tag="sm", bufs=2, name="t1")
                    nc.vector.tensor_mul(
                        t1[:],
                        x_sb[:, kc * SQ + qc * 512 : kc * SQ + (qc + 1) * 512],
                        A_p[:])
                    t2 = sb.tile([128, 512], F32, tag="sm", bufs=2, name="t2")
                    nc.vector.tensor_sub(t2[:], t1[:], B_p[:])
                    with nc.allow_low_precision(reason="f32r hT"):
                        nc.scalar.activation(
                            hT[:, kc * SQ + qc * 512 : kc * SQ + (qc + 1) * 512],
                            t2[:], AF.Identity,
                            bias=bias(_LNB, kc), scale=bias(_LNG, kc))

        # ---------------- phase D: FFN ----------------
        # W1/W2 slices loaded ONCE per rep (t-outer); per-t partial outputs
        # accumulate into SBUF via DVE so PSUM stays at 4 banks and weight
        # DMA halves vs the sc-outer variant.
        acc = sb.tile([128, NK * SQ], F32, tag="acc", name="acc")
        with (tc.tile_pool(name="ff_ps", bufs=4, space="PSUM") as ff_ps,
              tc.tile_pool(name="u_ps", bufs=3, space="PSUM") as u_ps):
            for t in range(NT):
                w1t = wtile(f"w1_{t}", [128, NK * DT], FR)
                nc.sync.dma_start(
                    out=w1t[:].rearrange("p (c m) -> p c m", m=DT),
                    in_=w1_d.rearrange("(c p) (t m) -> p c t m",
                                       p=128, t=NT)[:, :, t, :])
                w2t = wtile(f"w2_{t}", [128, MF * D], FR)
                nc.sync.dma_start(
                    out=w2t[:].rearrange("p (c m) -> p c m", m=D),
                    in_=w2_d.rearrange("(t c p) m -> p t c m",
                                       p=128, c=MF)[:, t])
                for sc in range(NQ):
                    ut = sb.tile([128, MF * 512], FR, tag="xu2", bufs=2, name="ut")
                    for mf in range(MF):
                        up = u_ps.tile([128, 512], F32, tag="up", name="up")
                        for kc in range(NK):
                            nc.tensor.matmul(
                                up[:],
                                w1t[:, kc * DT + mf * 128 : kc * DT + (mf + 1) * 128],
                                hT[:, kc * SQ + sc * 512 : kc * SQ + (sc + 1) * 512],
                                start=(kc == 0), stop=(kc == NK - 1))
                        with nc.allow_low_precision(reason="f32r u"):
                            nc.scalar.activation(
                                ut[:, mf * 512 : (mf + 1) * 512], up[:],
                                GELU_FUNC, bias=bias(_B1, t * MF + mf),
                                scale=1.0)
                    for md in range(NK):
                        ffp = ff_ps.tile([128, 512], F32, tag="ff", name="ffp")
                        for c in range(MF):
                            nc.tensor.matmul(
                                ffp[:],
                                w2t[:, c * D + md * 128 : c * D + (md + 1) * 128],
                                ut[:, c * 512 : (c + 1) * 512],
                                start=(c == 0), stop=(c == MF - 1))
                        a_sl = acc[:, md * SQ + sc * 512 : md * SQ + (sc + 1) * 512]
                        if t == 0:
                            nc.vector.tensor_copy(a_sl, ffp[:])
                        else:
                            nc.vector.tensor_add(a_sl, a_sl, ffp[:])
            for sc in range(NQ):
                for md in range(NK):
                    ot = sb.tile([128, 512], F32, tag="sm", bufs=2, name="ot")
                    nc.scalar.activation(
                        ot[:],
                        acc[:, md * SQ + sc * 512 : md * SQ + (sc + 1) * 512],
                        AF.Identity, bias=bias(_B2, md), scale=1.0)
                    nc.sync.dma_start(
                        out=r6(out_d)[:, md, sc * 512 : (sc + 1) * 512],
                        in_=ot[:])


def _build(reps=1):
    nc = bacc.Bacc("TRN2", target_bir_lowering=False, debug=False,
                   num_devices=N_CORES)
    io = {
        "xq8": nc.dram_tensor("xq8", [D, SQ], F8, kind="ExternalInput").ap(),
        "xqf": nc.dram_tensor("xqf", [D, SQ], F32, kind="ExternalInput").ap(),
        "xk8": nc.dram_tensor("xk8", [D, S], F8, kind="ExternalInput").ap(),
        "xv8": nc.dram_tensor("xv8", [D, S], F8, kind="ExternalInput").ap(),
        "wq8": nc.dram_tensor("wq8", [D, D], F8, kind="ExternalInput").ap(),
        "wk8": nc.dram_tensor("wk8", [D, D], F8, kind="ExternalInput").ap(),
        "wv8": nc.dram_tensor("wv8", [D, D], F8, kind="ExternalInput").ap(),
        "wo8": nc.dram_tensor("wo8", [D, D], F8, kind="ExternalInput").ap(),
        "sel65": nc.dram_tensor("sel65", [128, 64], FR,
                                kind="ExternalInput").ap(),
        "ones65": nc.dram_tensor("ones65", [65, 1024], FR,
                                 kind="ExternalInput").ap(),
        "w1": nc.dram_tensor("w1", [D, DF], FR, kind="ExternalInput").ap(),
        "w2": nc.dram_tensor("w2", [DF, D], FR, kind="ExternalInput").ap(),
        "sp": nc.dram_tensor("sp", [128, 828], F32, kind="ExternalInput").ap(),
        "ones_fr": nc.dram_tensor("ones_fr", [128, 128], FR,
                                  kind="ExternalInput").ap(),
        "out": nc.dram_tensor("out", [D, SQ], F32, kind="ExternalOutput").ap(),
    }
    with tile.TileContext(nc) as tc:
        if reps == 1:
            _body(nc, tc, io)
        else:
            with tc.For_i(0, reps, 1):
                _body(nc, tc, io)
    nc.compile()
    return nc


_NC = None


def _get_nc():
    global _NC
    if _NC is None:
        _NC = _build()
    return _NC


def make_in_maps(inputs):
    """Shard + lay out the full inputs for the 8 cores (numpy only)."""
    f = lambda k: np.asarray(inputs[k], np.float32)
    f8 = lambda a: np.ascontiguousarray(a).astype(ml_dtypes.float8_e4m3)
    Q, K, V = f("Q"), f("K"), f("V")
    sp = np.zeros((128, 828), np.float32)
    for idx, key in ((_BQ, "bq"), (_BK, "bk"), (_B2, "b2"),
                     (_LNG, "ln_g"), (_LNB, "ln_b")):
        sp[:, idx : idx + NK] = f(key).reshape(NK, 128).T
    sp[:, _B1 : _B1 + NF] = f("b1").reshape(NF, 128).T
    sp[:, _BV : _BV + D] = np.broadcast_to(f("bv"), (128, D))
    sel65 = np.zeros((128, 64), np.float32)
    sel65[64, :] = 64.0
    shared = {
        "wq8": f8(16.0 * f("Wq")), "wk8": f8(16.0 * f("Wk")),
        "wv8": f8(16.0 * f("Wv")), "wo8": f8(16.0 * f("Wo")),
        "w1": f("W1"), "w2": f("W2"), "sp": sp,
        "ones_fr": np.ones((128, 128), np.float32),
        "sel65": sel65,
        "ones65": np.ones((65, 1024), np.float32),
    }
    bo = f("bo")
    in_maps = []
    for c in range(N_CORES):
        b, half = divmod(c, 2)
        r0 = half * SQ
        xq = np.ascontiguousarray(Q[b, r0 : r0 + SQ, :].T)
        in_maps.append(dict(
            shared,
            xq8=f8(xq),
            xqf=xq + bo[:, None],
            xk8=f8(K[b].T),
            xv8=f8(V[b].T),
        ))
    return in_maps


def assemble(results):
    out = np.empty((B, S, D), np.float32)
    for c in range(N_CORES):
        b, half = divmod(c, 2)
        r0 = half * SQ
        out[b, r0 : r0 + SQ, :] = results[c]["out"].T
    return out


def kernel(**inputs):
    nc = _get_nc()
    res = run_bass_kernel_spmd(nc, make_in_maps(inputs), list(range(N_CORES)))
    return assemble(res.results)
